# revision 20
# baseline (speedup 1.0000x reference)
"""Trainium2 Bass kernel for nn_MemoryKernelModel — v5 (2-hop cycle).

Math: the rfft/einsum/irfft pipeline is a fixed 32-lag matrix FIR; the
400-step recurrence runs in blocks of T=8 steps, batch split 2 streams x
(2 groups x 256 cols). All update weights are pre-scaled by DT (v4).

v4 was latency-bound on the per-stream 4-hop dependency cycle
gelu -> mlpj -> evac -> h1 -> gelu (~1774ns/step, all engines <60% busy).
v5 shortens the cycle to 2 hops by building the NEXT step's gelu input
h(t) = W1'u_{t-1} directly in PSUM from three sources:
  - zg:   dense DT*(w2@W1) matmul from g_{t-1}   <- the only cycle hop
  - whin: in-block gather of older u rows with taps (DT*A_tau + I)'W1
  - zl:   W1' applied to the block-lag sum, which a SEPARATE Lh PSUM
          accumulates via duplicate whist matmuls (a=1..4) and a single
          per-block DVE copy to SBUF (lhW)
The constant DT*W1'b2 term rides the gelu bias (b1g2); the U side (pc,
wsc/mlpj/whist scatter + tensor_scalar evac + dtb2) is unchanged from v4.
Cycle ~= gelu(398+185) + zg(107+173) + sems ~= 920ns; PE is the new bound
at ~6 passes x 107ns x 2 streams ~= 1300ns/step.

axon op-set constraint (probed): tensor_tensor / scalar_tensor_tensor /
gpsimd.* do NOT compile; only matmul, transpose, activation,
tensor_scalar, tensor_copy, memset, dma_start are usable.
"""
# STATUS: VERIFIED — 680716 ns, rel err 4.548e-3 (full test.py, 8 cores).
# The earlier numerical failure was an emission-order race: block_boundary
# was emitted between tail(0) and tail(1), so stream 1's histL a=1 read
# its tile before stream 1's final evac was issued (Tile cannot order
# against not-yet-issued writes) -> stale s=7 taps. Fixed by per-stream
# boundary emission (boundary_stream after each stream's own tail).
# The weight algebra was confirmed correct by numpy emulation (emu_v5.py,
# max rel err 8.5e-5 over 32 steps).
#
# Remaining structure (~1700ns/step avg): mid-block cadence ~1530ns
# (PE ~1230ns busy/step over 10+ passes; gelu chain 613 + PE contention),
# block boundary adds ~170/step (last-evac -> histL a=1 -> lhev -> pcinj
# serial chain, ~1100ns x 50 blocks). lhW double-buffering was tried and
# changed nothing (the W-A-R was not binding). Next levers: shave a PE
# pass (whin/zl/wsc merges all blocked by differing rhs/out tiles), or
# overlap the boundary chain by splitting whistL[0] (s<=6 early / s=7
# late) — analysis says neither gains much.
# PROBED: the zg2 second-level expansion (whin sources 2-steps-stale +
# extra dense zg2 pass, breaking the evac->whin->gelu edge) was timing-
# probed with a structural dummy: 666808 ns, i.e. only ~2% — the added
# PE pass eats the chain savings. Not worth landing.
#
# Timeline-sim facts (see also kernel_v5_wip.py history):
#   - Tile dep-tracking is tile-granular for PSUM: never pack two
#     pipeline phases (e.g. h parities) into one tile.
#   - Engines complete out-of-order (sem waits precede queue entry);
#     issue order barely matters, data deps + busy time dominate.
#   - axon compiles only matmul/transpose/activation/tensor_scalar/
#     tensor_copy/memset/dma; tensor_tensor, scalar_tensor_tensor and
#     all gpsimd ops fail.
import os
import numpy as np

import concourse.bacc as bacc_mod
import concourse.tile as tile
import concourse.mybir as mybir
from concourse.bass_utils import run_bass_kernel_spmd

B, H_HIST, C, S, HIDDEN = 8192, 512, 8, 32, 64
DT = 0.05
STEPS = 400
T = 8
NCORES = 8
BPC = B // NCORES
NSTREAM = 2
NCOL = 256
F32 = mybir.dt.float32
F32R = mybir.dt.float32r


# ----------------------------------------------------------------------------
# host-side math
# ----------------------------------------------------------------------------

def compute_lag_matrices(freq_w_real, freq_w_imag):
    fw = freq_w_real.astype(np.float64) + 1j * freq_w_imag.astype(np.float64)
    basis = np.zeros((S * C, S, C))
    idx = np.arange(S * C)
    basis[idx, idx // C, idx % C] = 1.0
    hft = np.fft.rfft(basis, axis=1)
    oft = np.einsum('bfc,ocf->bfo', hft, fw)
    k = np.fft.irfft(oft, n=S, axis=1)[:, -1, :]
    M = np.zeros((C, C, S))
    M[:, idx % C, idx // C] = k.T
    return np.stack([M[:, :, S - 1 - tau] for tau in range(S)])


def build_weights(A, w1, b1, w2, b2):
    """Returns dict of 128x128 weight rounds + bias vectors.

    Row convention for u-space tiles: row 16*s + 8*g + c.
    Hidden-space rows: 64*g + h. kterm = A_tau @ u (A[tau][dst, src]).
    lhsT convention: out[p, f] = sum_k lhsT[k, p] * rhs[k, f].
    """
    A = A.astype(np.float64)
    w1 = w1.astype(np.float64)
    w2 = w2.astype(np.float64)
    b1 = b1.astype(np.float64)
    b2 = b2.astype(np.float64)

    def ublk(s, g):
        return slice(16 * s + 8 * g, 16 * s + 8 * g + 8)

    def hblk(g):
        return slice(64 * g, 64 * g + 64)

    # pc-side whist (a=1..4): tile n-a source s -> pc target j, tau=8a+j-s-1
    whist = np.zeros((4, 128, 128))
    for a in range(1, 5):
        for s_ in range(T):
            if s_ > 8 * a - 2:
                continue
            for j in range(T):
                tau = 8 * a + j - s_ - 1
                if tau <= 31:
                    for g in range(2):
                        whist[a - 1][ublk(s_, g), ublk(j, g)] = DT * A[tau].T
    # Lh-side whistL: same, but a=1 extended with source s=7 (prev row 7),
    # taps tau=j for targets j>=1 (target 0's I+DT*A0 lives in whin[1]).
    whistL = whist.copy()
    for j in range(1, T):
        for g in range(2):
            whistL[0][ublk(7, g), ublk(j, g)] = DT * A[j].T

    # wsc[j]: in-block scatter, source row (j-1)%8 (prev tile row 7 for j=0),
    # targets jp=j..7, taps DT*A[jp-j] + I[jp==j]  (v4, unchanged)
    wsc = np.zeros((T, 128, 128))
    wmlpj = np.zeros((T, 128, 128))
    for j in range(T):
        ss = (j - 1) % 8
        for g in range(2):
            # j=0: only the jp=0 block; prev-row-7 taps for jp>=1 now reach
            # pc through whistL[0]'s s=7 extension via the lhW identity inject
            for jp in range(j, T if j > 0 else 1):
                blk = DT * A[jp - j].T
                if jp == j:
                    blk = blk + np.eye(C)
                wsc[j][ublk(ss, g), ublk(jp, g)] = blk
            wmlpj[j][hblk(g), ublk(j, g)] = DT * w2

    # whin[t], t=1..8: gather for h(t) = W1' u_{t-1}: in-block sources.
    # t=1: prev-tile row 7 with (DT*A0+I)'W1; t>=2: tb rows s=0..t-2 with
    # (DT*A[t-2-s] + I[s==t-2])' W1.
    whin = np.zeros((T + 1, 128, 128))
    for g in range(2):
        whin[1][ublk(7, g), hblk(g)] = (DT * A[0] + np.eye(C)).T @ w1
    for t in range(2, T + 1):
        for s_ in range(t - 1):
            M = DT * A[t - 2 - s_]
            if s_ == t - 2:
                M = M + np.eye(C)
            for g in range(2):
                whin[t][ublk(s_, g), hblk(g)] = M.T @ w1

    # zl[t]: lhW row t-1 -> h rows via W1
    zl = np.zeros((T + 1, 128, 128))
    for t in range(1, T + 1):
        for g in range(2):
            zl[t][ublk(t - 1, g), hblk(g)] = w1

    # zg: dense DT*(w2@W1) per group
    wzg = np.zeros((128, 128))
    for g in range(2):
        wzg[hblk(g), hblk(g)] = DT * (w2 @ w1)

    # h1 init: u-row 7 -> hidden (for the very first gelu)
    wh1i = np.zeros((128, 128))
    for g in range(2):
        wh1i[ublk(7, g), hblk(g)] = w1

    b1g = np.tile(b1, 2).reshape(128, 1)
    b1g2 = np.tile(b1 + DT * (w1.T @ b2), 2).reshape(128, 1)
    dtb2 = np.tile(DT * b2, 16).reshape(128, 1)
    rounds = ([whistL[a] for a in range(4)] +
              [wsc[j] for j in range(T)] + [wmlpj[j] for j in range(T)] +
              [whin[t] for t in range(1, T + 1)] +
              [zl[t] for t in range(1, T + 1)] +
              [wzg, wh1i, np.eye(128)])
    wall = np.concatenate([r for r in rounds], axis=1).astype(np.float32)
    return wall, b1g.astype(np.float32), b1g2.astype(np.float32), \
        dtb2.astype(np.float32)


NW = 4 + 8 + 8 + 8 + 8 + 3  # 39 rounds


def init_history(hist_raw):
    Hn = hist_raw.shape[1]
    idx = np.linspace(0.0, Hn - 1.0, S)
    f = np.clip(np.floor(idx), 0, Hn - 1).astype(np.int64)
    c = np.clip(np.ceil(idx), 0, Hn - 1).astype(np.int64)
    w = (idx - np.floor(idx)).astype(np.float32)[None, :, None]
    hr = np.asarray(hist_raw, np.float32)
    return (1.0 - w) * hr[:, f, :] + w * hr[:, c, :]


def pack_uh0(hist0_core):
    out = np.zeros((NSTREAM, 4, 128, NCOL), np.float32)
    h = hist0_core.reshape(NSTREAM, 2, NCOL, S, C)
    for sg in range(NSTREAM):
        for m in range(4):
            for s_ in range(T):
                for g in range(2):
                    out[sg, m, 16 * s_ + 8 * g:16 * s_ + 8 * g + 8, :] = \
                        h[sg, g, :, 8 * m + s_, :].T
    return out


# ----------------------------------------------------------------------------
# device program
# ----------------------------------------------------------------------------

def build_nc(steps=STEPS):
    nblk = steps // T
    nc = bacc_mod.Bacc(None, target_bir_lowering=False)

    wall_d = nc.dram_tensor("wall", [128, NW * 128], F32, kind="ExternalInput")
    uh0w_d = nc.dram_tensor("uh0w", [128, NSTREAM * 4 * NCOL], F32, kind="ExternalInput")
    b1g_d = nc.dram_tensor("b1g", [128, 1], F32, kind="ExternalInput")
    b1g2_d = nc.dram_tensor("b1g2", [128, 1], F32, kind="ExternalInput")
    dtb2_d = nc.dram_tensor("dtb2", [128, 1], F32, kind="ExternalInput")
    traj_d = nc.dram_tensor("traj", [nblk, NSTREAM, 128, NCOL], F32R,
                            kind="ExternalOutput")

    with tile.TileContext(nc) as tc:
        with (
            tc.tile_pool(name="wtmp", bufs=2) as wtmp,
            tc.tile_pool(name="wpool", bufs=1) as wpool,
            tc.tile_pool(name="state", bufs=1) as state,
            tc.tile_pool(name="ps", bufs=1, space="PSUM") as ps,
        ):
            wall_st = wtmp.tile([128, NW * 128], F32, name="wall_st", tag="wall_st")
            nc.sync.dma_start(out=wall_st, in_=wall_d[:, :])
            uh0_st = wtmp.tile([128, NSTREAM * 4 * NCOL], F32, name="uh0_st", tag="uh0_st")
            nc.sync.dma_start(out=uh0_st, in_=uh0w_d[:, :])

            def load_round(i, tag):
                t = wpool.tile([128, 128], F32R, name=tag, tag=tag)
                nc.vector.tensor_copy(t, wall_st[:, i * 128:(i + 1) * 128])
                return t

            whistL_t = [load_round(a, f"whistL{a}") for a in range(4)]
            wsc_t = [load_round(4 + j, f"wsc{j}") for j in range(T)]
            wmlpj_t = [load_round(12 + j, f"wmlpj{j}") for j in range(T)]
            whin_t = [load_round(20 + t_, f"whin{t_}") for t_ in range(T)]  # whin[t=1..8]
            zl_t = [load_round(28 + t_, f"zl{t_}") for t_ in range(T)]      # zl[t=1..8]
            wzg_t = load_round(36, "wzg")
            wh1i_t = load_round(37, "wh1i")
            wid_t = load_round(38, "wident")
            b1g_t = wpool.tile([128, 1], F32, tag="b1g")
            nc.sync.dma_start(out=b1g_t, in_=b1g_d[:, :])
            b1g2_t = wpool.tile([128, 1], F32, tag="b1g2")
            nc.sync.dma_start(out=b1g2_t, in_=b1g2_d[:, :])
            dtb2_t = wpool.tile([128, 1], F32, tag="dtb2")
            nc.sync.dma_start(out=dtb2_t, in_=dtb2_d[:, :])

            uh = [[state.tile([128, NCOL], F32R, name=f"uh_{sg}_{i}", tag=f"uh_{sg}_{i}")
                   for i in range(6)] for sg in range(NSTREAM)]
            g_sb = [[state.tile([128, NCOL], F32R, name=f"g_{sg}_{p}", tag=f"g_{sg}_{p}")
                     for p in range(2)] for sg in range(NSTREAM)]
            lhW = [state.tile([128, NCOL], F32R, name=f"lhW_{sg}", tag=f"lhW_{sg}")
                   for sg in range(NSTREAM)]
            # PSUM: pc 2x2 banks, Lh 2 banks, h 2 banks (2 halves in cols)
            p_ps = [ps.tile([128, NCOL], F32, name=f"p_{sg}", tag=f"p_{sg}")
                    for sg in range(NSTREAM)]
            lh_ps = [ps.tile([128, NCOL], F32, name=f"lh_{sg}", tag=f"lh_{sg}")
                     for sg in range(NSTREAM)]
            h_ps = [[ps.tile([128, NCOL], F32, name=f"h_{sg}_{p}", tag=f"h_{sg}_{p}")
                     for p in range(2)] for sg in range(NSTREAM)]

            for sg in range(NSTREAM):
                for m in range(4):
                    off = (sg * 4 + m) * NCOL
                    nc.vector.tensor_copy(uh[sg][m], uh0_st[:, off:off + NCOL])

            def emit_pcinj(sg):
                nc.tensor.matmul(p_ps[sg][:, :], wid_t[:], lhW[sg][:],
                                 start=True, stop=False)

            def emit_histL(n, sg, a):
                nc.tensor.matmul(
                    lh_ps[sg][:, :], whistL_t[a - 1][:],
                    uh[sg][(n - a + 4) % 6][:],
                    start=(a == 4), stop=(a == 1),
                )

            def emit_lhev(sg):
                nc.vector.tensor_copy(lhW[sg], lh_ps[sg][:, :])

            def emit_out(n, sg):
                nc.sync.dma_start(
                    out=traj_d[n, sg, :, :],
                    in_=uh[sg][(n + 4) % 6][:, :],
                )

            # ---- flat software pipeline over global steps ----
            # PE order per step (true time order; per-engine sems are
            # ordinal, so any mis-ordered gated op couples its gate into
            # every later wait): tail(0,gs) | front(1,gs) | tail(1,gs) |
            # front(0,gs+1).
            extras = []

            def pop_extras(k):
                for _ in range(k):
                    if not extras:
                        return
                    e = extras.pop(0)
                    if e[0] == "histL":
                        emit_histL(e[1], e[2], e[3])
                    else:
                        emit_out(e[1], e[2])

            def emit_front(sg, gs):
                # wsc/whin/zl for step gs (gated on evac_sg(gs-1)) + gelu(gs)
                n, j = divmod(gs, T)
                if gs >= steps:
                    return
                pc = p_ps[sg]
                tb = (n + 4) % 6
                last = (gs == steps - 1)
                rhs_prev = uh[sg][(n + 3) % 6][:, :]
                rhs = rhs_prev if j == 0 else uh[sg][tb][:, :]
                hhalf = h_ps[sg][(j + 1) % 2][:, :]
                nc.tensor.matmul(pc[:, :], wsc_t[j][:], rhs,
                                 start=False, stop=False)
                if not last:
                    nc.tensor.matmul(hhalf, whin_t[j][:], rhs,
                                     start=True, stop=False)
                    nc.tensor.matmul(hhalf, zl_t[j][:], lhW[sg][:],
                                     start=False, stop=False)
                bias = b1g_t if gs == 0 else b1g2_t
                nc.scalar.activation(
                    g_sb[sg][gs % 2], h_ps[sg][j % 2][:, :],
                    mybir.ActivationFunctionType.Gelu,
                    bias=bias[:], scale=1.0,
                )

            def emit_tail(sg, gs):
                # zg/mlpj (gated on gelu_sg(gs)) + evac
                n, j = divmod(gs, T)
                pc = p_ps[sg]
                tb = (n + 4) % 6
                last = (gs == steps - 1)
                if not last:
                    nc.tensor.matmul(h_ps[sg][(j + 1) % 2][:, :], wzg_t[:],
                                     g_sb[sg][gs % 2][:], start=False, stop=True)
                nc.tensor.matmul(pc[:, :], wmlpj_t[j][:], g_sb[sg][gs % 2][:],
                                 start=False, stop=(j == T - 1))
                ub = 32 * (j // 2)
                nc.vector.tensor_scalar(
                    out=uh[sg][tb][ub:ub + 32, :],
                    in0=pc[ub:ub + 32, :],
                    scalar1=1.0, scalar2=dtb2_t[ub:ub + 32],
                    op0=mybir.AluOpType.mult, op1=mybir.AluOpType.add,
                )

            def boundary_stream(sg, n):
                # entering block n for stream sg: MUST be emitted after
                # tail(sg, 8n-1) so histL a=1 sees the final evac of tile
                # n-1 (Tile cannot order against not-yet-issued writes).
                emit_histL(n, sg, 1)
                emit_lhev(sg)
                emit_pcinj(sg)

            def queue_boundary_extras(n):
                if n + 1 < nblk:
                    for a in (4, 3, 2):
                        for sg in range(NSTREAM):
                            extras.append(("histL", n + 1, sg, a))
                for sg in range(NSTREAM):
                    extras.append(("out", n - 1, sg))

            # prologue: block 0 prefetch + first front
            for a in (4, 3, 2, 1):
                for sg in range(NSTREAM):
                    emit_histL(0, sg, a)
            for sg in range(NSTREAM):
                emit_lhev(sg)
            for sg in range(NSTREAM):
                emit_pcinj(sg)
                nc.tensor.matmul(h_ps[sg][0][:, :], wh1i_t[:],
                                 uh[sg][3][:], start=True, stop=True)
            if nblk > 1:
                for a in (4, 3, 2):
                    for sg in range(NSTREAM):
                        extras.append(("histL", 1, sg, a))
            emit_front(0, 0)

            emit_front(1, 0)
            for gs in range(steps):
                boundary = gs + 1 < steps and (gs + 1) % T == 0
                emit_tail(0, gs)
                if boundary:
                    boundary_stream(0, (gs + 1) // T)
                emit_front(0, gs + 1)
                pop_extras(1)
                emit_tail(1, gs)
                if boundary:
                    boundary_stream(1, (gs + 1) // T)
                    queue_boundary_extras((gs + 1) // T)
                emit_front(1, gs + 1)
                pop_extras(1)
            pop_extras(len(extras))

            for sg in range(NSTREAM):
                emit_out(nblk - 1, sg)
    nc.compile()
    return nc


# ----------------------------------------------------------------------------
# entry point
# ----------------------------------------------------------------------------

_NC_CACHE = {}


def kernel(hist_raw, freq_w_real, freq_w_imag, w1, b1, w2, b2):
    hist_raw = np.asarray(hist_raw, np.float32)
    A = compute_lag_matrices(np.asarray(freq_w_real), np.asarray(freq_w_imag))
    wall, b1g, b1g2, dtb2 = build_weights(
        A, np.asarray(w1), np.asarray(b1), np.asarray(w2), np.asarray(b2))
    hist0 = init_history(hist_raw)

    if "nc" not in _NC_CACHE:
        _NC_CACHE["nc"] = build_nc(STEPS)
    nc = _NC_CACHE["nc"]

    in_maps = []
    for core in range(NCORES):
        hc = hist0[core * BPC:(core + 1) * BPC]
        uh0 = pack_uh0(hc)          # (NSTREAM, 4, 128, NCOL)
        uh0w = uh0.transpose(2, 0, 1, 3).reshape(128, -1).astype(np.float32)
        in_maps.append({
            "uh0w": uh0w, "wall": wall, "b1g": b1g, "b1g2": b1g2,
            "dtb2": dtb2,
        })

    trace = os.environ.get("KERNEL_TRACE", "0") == "1"
    try:
        res = run_bass_kernel_spmd(nc, in_maps, core_ids=list(range(NCORES)),
                                   trace=trace)
    except ModuleNotFoundError:
        res = run_bass_kernel_spmd(nc, in_maps, core_ids=list(range(NCORES)))
    _NC_CACHE["last_result"] = res
    if trace and getattr(res, "exec_time_ns", None):
        print(f"HW exec time: {res.exec_time_ns} ns")
    nblk = STEPS // T
    outs = []
    for r in res.results:
        d = np.asarray(r["traj"]).view(np.float32).reshape(
            nblk, NSTREAM, T, 2, C, NCOL)
        # traj[sg*512 + g*256 + col, 8n+s, c] = d[n, sg, s, g, c, col]
        t = d.transpose(1, 3, 5, 0, 2, 4).reshape(BPC, STEPS, C)
        outs.append(t)
    return np.concatenate(outs, axis=0)


# revision 23
# speedup vs baseline: 1.0188x; 1.0188x over previous
"""Trainium2 Bass kernel for nn_MemoryKernelModel — v5 (2-hop cycle).

Math: the rfft/einsum/irfft pipeline is a fixed 32-lag matrix FIR; the
400-step recurrence runs in blocks of T=8 steps, batch split 2 streams x
(2 groups x 256 cols). All update weights are pre-scaled by DT (v4).

v4 was latency-bound on the per-stream 4-hop dependency cycle
gelu -> mlpj -> evac -> h1 -> gelu (~1774ns/step, all engines <60% busy).
v5 shortens the cycle to 2 hops by building the NEXT step's gelu input
h(t) = W1'u_{t-1} directly in PSUM from three sources:
  - zg:   dense DT*(w2@W1) matmul from g_{t-1}   <- the only cycle hop
  - whin: in-block gather of older u rows with taps (DT*A_tau + I)'W1
  - zl:   W1' applied to the block-lag sum, which a SEPARATE Lh PSUM
          accumulates via duplicate whist matmuls (a=1..4) and a single
          per-block DVE copy to SBUF (lhW)
The constant DT*W1'b2 term rides the gelu bias (b1g2); the U side (pc,
wsc/mlpj/whist scatter + tensor_scalar evac + dtb2) is unchanged from v4.
Cycle ~= gelu(398+185) + zg(107+173) + sems ~= 920ns; PE is the new bound
at ~6 passes x 107ns x 2 streams ~= 1300ns/step.

axon op-set constraint (probed): tensor_tensor / scalar_tensor_tensor /
gpsimd.* do NOT compile; only matmul, transpose, activation,
tensor_scalar, tensor_copy, memset, dma_start are usable.
"""
# STATUS: VERIFIED — 680716 ns, rel err 4.548e-3 (full test.py, 8 cores).
# The earlier numerical failure was an emission-order race: block_boundary
# was emitted between tail(0) and tail(1), so stream 1's histL a=1 read
# its tile before stream 1's final evac was issued (Tile cannot order
# against not-yet-issued writes) -> stale s=7 taps. Fixed by per-stream
# boundary emission (boundary_stream after each stream's own tail).
# The weight algebra was confirmed correct by numpy emulation (emu_v5.py,
# max rel err 8.5e-5 over 32 steps).
#
# Remaining structure (~1700ns/step avg): mid-block cadence ~1530ns
# (PE ~1230ns busy/step over 10+ passes; gelu chain 613 + PE contention),
# block boundary adds ~170/step (last-evac -> histL a=1 -> lhev -> pcinj
# serial chain, ~1100ns x 50 blocks). lhW double-buffering was tried and
# changed nothing (the W-A-R was not binding). Next levers: shave a PE
# pass (whin/zl/wsc merges all blocked by differing rhs/out tiles), or
# overlap the boundary chain by splitting whistL[0] (s<=6 early / s=7
# late) — analysis says neither gains much.
# PROBED: the zg2 second-level expansion (whin sources 2-steps-stale +
# extra dense zg2 pass, breaking the evac->whin->gelu edge) was timing-
# probed with a structural dummy: 666808 ns, i.e. only ~2% — the added
# PE pass eats the chain savings. Not worth landing.
#
# Timeline-sim facts (see also kernel_v5_wip.py history):
#   - Tile dep-tracking is tile-granular for PSUM: never pack two
#     pipeline phases (e.g. h parities) into one tile.
#   - Engines complete out-of-order (sem waits precede queue entry);
#     issue order barely matters, data deps + busy time dominate.
#   - axon compiles only matmul/transpose/activation/tensor_scalar/
#     tensor_copy/memset/dma; tensor_tensor, scalar_tensor_tensor and
#     all gpsimd ops fail.
import os
import numpy as np

import concourse.bacc as bacc_mod
import concourse.tile as tile
import concourse.mybir as mybir
from concourse.bass_utils import run_bass_kernel_spmd

B, H_HIST, C, S, HIDDEN = 8192, 512, 8, 32, 64
DT = 0.05
STEPS = 400
T = 8
NCORES = 8
BPC = B // NCORES
NSTREAM = 2
NCOL = 256
F32 = mybir.dt.float32
F32R = mybir.dt.float32r


# ----------------------------------------------------------------------------
# host-side math
# ----------------------------------------------------------------------------

def compute_lag_matrices(freq_w_real, freq_w_imag):
    fw = freq_w_real.astype(np.float64) + 1j * freq_w_imag.astype(np.float64)
    basis = np.zeros((S * C, S, C))
    idx = np.arange(S * C)
    basis[idx, idx // C, idx % C] = 1.0
    hft = np.fft.rfft(basis, axis=1)
    oft = np.einsum('bfc,ocf->bfo', hft, fw)
    k = np.fft.irfft(oft, n=S, axis=1)[:, -1, :]
    M = np.zeros((C, C, S))
    M[:, idx % C, idx // C] = k.T
    return np.stack([M[:, :, S - 1 - tau] for tau in range(S)])


def build_weights(A, w1, b1, w2, b2):
    """Returns dict of 128x128 weight rounds + bias vectors.

    Row convention for u-space tiles: row 16*s + 8*g + c.
    Hidden-space rows: 64*g + h. kterm = A_tau @ u (A[tau][dst, src]).
    lhsT convention: out[p, f] = sum_k lhsT[k, p] * rhs[k, f].
    """
    A = A.astype(np.float64)
    w1 = w1.astype(np.float64)
    w2 = w2.astype(np.float64)
    b1 = b1.astype(np.float64)
    b2 = b2.astype(np.float64)

    def ublk(s, g):
        return slice(16 * s + 8 * g, 16 * s + 8 * g + 8)

    def hblk(g):
        return slice(64 * g, 64 * g + 64)

    # pc-side whist (a=1..4): tile n-a source s -> pc target j, tau=8a+j-s-1
    whist = np.zeros((4, 128, 128))
    for a in range(1, 5):
        for s_ in range(T):
            if s_ > 8 * a - 2:
                continue
            for j in range(T):
                tau = 8 * a + j - s_ - 1
                if tau <= 31:
                    for g in range(2):
                        whist[a - 1][ublk(s_, g), ublk(j, g)] = DT * A[tau].T
    # Lh-side whistL: same, but a=1 extended with source s=7 (prev row 7),
    # taps tau=j for targets j>=1 (target 0's I+DT*A0 lives in whin[1]).
    whistL = whist.copy()
    for j in range(1, T):
        for g in range(2):
            whistL[0][ublk(7, g), ublk(j, g)] = DT * A[j].T

    # wsc[j]: in-block scatter, source row (j-1)%8 (prev tile row 7 for j=0),
    # targets jp=j..7, taps DT*A[jp-j] + I[jp==j]  (v4, unchanged)
    wsc = np.zeros((T, 128, 128))
    wmlpj = np.zeros((T, 128, 128))
    for j in range(T):
        ss = (j - 1) % 8
        for g in range(2):
            # j=0: only the jp=0 block; prev-row-7 taps for jp>=1 now reach
            # pc through whistL[0]'s s=7 extension via the lhW identity inject
            for jp in range(j, T if j > 0 else 1):
                blk = DT * A[jp - j].T
                if jp == j:
                    blk = blk + np.eye(C)
                wsc[j][ublk(ss, g), ublk(jp, g)] = blk
            wmlpj[j][hblk(g), ublk(j, g)] = DT * w2

    # whin[t], t=1..8: gather for h(t) = W1' u_{t-1}: in-block sources.
    # t=1: prev-tile row 7 with (DT*A0+I)'W1; t>=2: tb rows s=0..t-2 with
    # (DT*A[t-2-s] + I[s==t-2])' W1.
    whin = np.zeros((T + 1, 128, 128))
    for g in range(2):
        whin[1][ublk(7, g), hblk(g)] = (DT * A[0] + np.eye(C)).T @ w1
    for t in range(2, T + 1):
        for s_ in range(t - 1):
            M = DT * A[t - 2 - s_]
            if s_ == t - 2:
                M = M + np.eye(C)
            for g in range(2):
                whin[t][ublk(s_, g), hblk(g)] = M.T @ w1

    # zl[t]: lhW row t-1 -> h rows via W1
    zl = np.zeros((T + 1, 128, 128))
    for t in range(1, T + 1):
        for g in range(2):
            zl[t][ublk(t - 1, g), hblk(g)] = w1

    # zg: dense DT*(w2@W1) per group
    wzg = np.zeros((128, 128))
    for g in range(2):
        wzg[hblk(g), hblk(g)] = DT * (w2 @ w1)

    # h1 init: u-row 7 -> hidden (for the very first gelu)
    wh1i = np.zeros((128, 128))
    for g in range(2):
        wh1i[ublk(7, g), hblk(g)] = w1

    b1g = np.tile(b1, 2).reshape(128, 1)
    b1g2 = np.tile(b1 + DT * (w1.T @ b2), 2).reshape(128, 1)
    dtb2 = np.tile(DT * b2, 16).reshape(128, 1)
    rounds = ([whistL[a] for a in range(4)] +
              [wsc[j] for j in range(T)] + [wmlpj[j] for j in range(T)] +
              [whin[t] for t in range(1, T + 1)] +
              [zl[t] for t in range(1, T + 1)] +
              [wzg, wh1i, np.eye(128)])
    wall = np.concatenate([r for r in rounds], axis=1).astype(np.float32)
    return wall, b1g.astype(np.float32), b1g2.astype(np.float32), \
        dtb2.astype(np.float32)


NW = 4 + 8 + 8 + 8 + 8 + 3  # 39 rounds


def init_history(hist_raw):
    Hn = hist_raw.shape[1]
    idx = np.linspace(0.0, Hn - 1.0, S)
    f = np.clip(np.floor(idx), 0, Hn - 1).astype(np.int64)
    c = np.clip(np.ceil(idx), 0, Hn - 1).astype(np.int64)
    w = (idx - np.floor(idx)).astype(np.float32)[None, :, None]
    hr = np.asarray(hist_raw, np.float32)
    return (1.0 - w) * hr[:, f, :] + w * hr[:, c, :]


def pack_uh0(hist0_core):
    out = np.zeros((NSTREAM, 4, 128, NCOL), np.float32)
    h = hist0_core.reshape(NSTREAM, 2, NCOL, S, C)
    for sg in range(NSTREAM):
        for m in range(4):
            for s_ in range(T):
                for g in range(2):
                    out[sg, m, 16 * s_ + 8 * g:16 * s_ + 8 * g + 8, :] = \
                        h[sg, g, :, 8 * m + s_, :].T
    return out


# ----------------------------------------------------------------------------
# device program
# ----------------------------------------------------------------------------

def build_nc(steps=STEPS):
    nblk = steps // T
    nc = bacc_mod.Bacc(None, target_bir_lowering=False)

    wall_d = nc.dram_tensor("wall", [128, NW * 128], F32, kind="ExternalInput")
    uh0w_d = nc.dram_tensor("uh0w", [128, NSTREAM * 4 * NCOL], F32, kind="ExternalInput")
    b1g_d = nc.dram_tensor("b1g", [128, 1], F32, kind="ExternalInput")
    b1g2_d = nc.dram_tensor("b1g2", [128, 1], F32, kind="ExternalInput")
    dtb2_d = nc.dram_tensor("dtb2", [128, 1], F32, kind="ExternalInput")
    traj_d = nc.dram_tensor("traj", [nblk, NSTREAM, 128, NCOL], F32R,
                            kind="ExternalOutput")

    with tile.TileContext(nc) as tc:
        with (
            tc.tile_pool(name="wtmp", bufs=2) as wtmp,
            tc.tile_pool(name="wpool", bufs=1) as wpool,
            tc.tile_pool(name="state", bufs=1) as state,
            tc.tile_pool(name="ps", bufs=1, space="PSUM") as ps,
        ):
            wall_st = wtmp.tile([128, NW * 128], F32, name="wall_st", tag="wall_st")
            nc.sync.dma_start(out=wall_st, in_=wall_d[:, :])
            uh0_st = wtmp.tile([128, NSTREAM * 4 * NCOL], F32, name="uh0_st", tag="uh0_st")
            nc.sync.dma_start(out=uh0_st, in_=uh0w_d[:, :])

            def load_round(i, tag):
                t = wpool.tile([128, 128], F32R, name=tag, tag=tag)
                nc.vector.tensor_copy(t, wall_st[:, i * 128:(i + 1) * 128])
                return t

            whistL_t = [load_round(a, f"whistL{a}") for a in range(4)]
            wsc_t = [load_round(4 + j, f"wsc{j}") for j in range(T)]
            wmlpj_t = [load_round(12 + j, f"wmlpj{j}") for j in range(T)]
            whin_t = [load_round(20 + t_, f"whin{t_}") for t_ in range(T)]  # whin[t=1..8]
            zl_t = [load_round(28 + t_, f"zl{t_}") for t_ in range(T)]      # zl[t=1..8]
            wzg_t = load_round(36, "wzg")
            wh1i_t = load_round(37, "wh1i")
            wid_t = load_round(38, "wident")
            b1g_t = wpool.tile([128, 1], F32, tag="b1g")
            nc.sync.dma_start(out=b1g_t, in_=b1g_d[:, :])
            b1g2_t = wpool.tile([128, 1], F32, tag="b1g2")
            nc.sync.dma_start(out=b1g2_t, in_=b1g2_d[:, :])
            dtb2_t = wpool.tile([128, 1], F32, tag="dtb2")
            nc.sync.dma_start(out=dtb2_t, in_=dtb2_d[:, :])

            uh = [[state.tile([128, NCOL], F32R, name=f"uh_{sg}_{i}", tag=f"uh_{sg}_{i}")
                   for i in range(6)] for sg in range(NSTREAM)]
            g_sb = [[state.tile([128, NCOL], F32R, name=f"g_{sg}_{p}", tag=f"g_{sg}_{p}")
                     for p in range(2)] for sg in range(NSTREAM)]
            lhW = [state.tile([128, NCOL], F32R, name=f"lhW_{sg}", tag=f"lhW_{sg}")
                   for sg in range(NSTREAM)]
            # PSUM: pc 2x2 banks, Lh 2 banks, h 2 banks (2 halves in cols)
            p_ps = [ps.tile([128, NCOL], F32, name=f"p_{sg}", tag=f"p_{sg}")
                    for sg in range(NSTREAM)]
            lh_ps = [ps.tile([128, NCOL], F32, name=f"lh_{sg}", tag=f"lh_{sg}")
                     for sg in range(NSTREAM)]
            h_ps = [[ps.tile([128, NCOL], F32, name=f"h_{sg}_{p}", tag=f"h_{sg}_{p}")
                     for p in range(2)] for sg in range(NSTREAM)]

            for sg in range(NSTREAM):
                for m in range(4):
                    off = (sg * 4 + m) * NCOL
                    nc.vector.tensor_copy(uh[sg][m], uh0_st[:, off:off + NCOL])

            def emit_pcinj(sg):
                nc.tensor.matmul(p_ps[sg][:, :], wid_t[:], lhW[sg][:],
                                 start=False, stop=False)

            def emit_histL(n, sg, a):
                nc.tensor.matmul(
                    lh_ps[sg][:, :], whistL_t[a - 1][:],
                    uh[sg][(n - a + 4) % 6][:],
                    start=(a == 4), stop=(a == 1),
                )

            def emit_lhev(sg):
                nc.vector.tensor_copy(lhW[sg], lh_ps[sg][:, :])

            def emit_out(n, sg):
                nc.sync.dma_start(
                    out=traj_d[n, sg, :, :],
                    in_=uh[sg][(n + 4) % 6][:, :],
                )

            # ---- flat software pipeline over global steps ----
            # PE order per step (true time order; per-engine sems are
            # ordinal, so any mis-ordered gated op couples its gate into
            # every later wait): tail(0,gs) | front(1,gs) | tail(1,gs) |
            # front(0,gs+1).
            extras = []

            def pop_extras(k):
                for _ in range(k):
                    if not extras:
                        return
                    e = extras.pop(0)
                    if e[0] == "histL":
                        emit_histL(e[1], e[2], e[3])
                    else:
                        emit_out(e[1], e[2])

            def emit_front(sg, gs):
                # wsc/whin/zl for step gs (gated on evac_sg(gs-1)) + gelu(gs)
                n, j = divmod(gs, T)
                if gs >= steps:
                    return
                pc = p_ps[sg]
                tb = (n + 4) % 6
                last = (gs == steps - 1)
                rhs_prev = uh[sg][(n + 3) % 6][:, :]
                rhs = rhs_prev if j == 0 else uh[sg][tb][:, :]
                hhalf = h_ps[sg][(j + 1) % 2][:, :]
                nc.tensor.matmul(pc[:, :], wsc_t[j][:], rhs,
                                 start=(j == 0), stop=False)
                if not last:
                    nc.tensor.matmul(hhalf, whin_t[j][:], rhs,
                                     start=True, stop=False)
                    nc.tensor.matmul(hhalf, zl_t[j][:], lhW[sg][:],
                                     start=False, stop=False)
                bias = b1g_t if gs == 0 else b1g2_t
                nc.scalar.activation(
                    g_sb[sg][gs % 2], h_ps[sg][j % 2][:, :],
                    mybir.ActivationFunctionType.Gelu,
                    bias=bias[:], scale=1.0,
                )

            def emit_tail(sg, gs):
                # zg/mlpj (gated on gelu_sg(gs)) + evac
                n, j = divmod(gs, T)
                pc = p_ps[sg]
                tb = (n + 4) % 6
                last = (gs == steps - 1)
                if not last:
                    nc.tensor.matmul(h_ps[sg][(j + 1) % 2][:, :], wzg_t[:],
                                     g_sb[sg][gs % 2][:], start=False, stop=True)
                nc.tensor.matmul(pc[:, :], wmlpj_t[j][:], g_sb[sg][gs % 2][:],
                                 start=False, stop=(j == T - 1))
                ub = 32 * (j // 2)
                nc.vector.tensor_scalar(
                    out=uh[sg][tb][ub:ub + 32, :],
                    in0=pc[ub:ub + 32, :],
                    scalar1=1.0, scalar2=dtb2_t[ub:ub + 32],
                    op0=mybir.AluOpType.mult, op1=mybir.AluOpType.add,
                )

            def boundary_stream(sg, n):
                # entering block n for stream sg: MUST be emitted after
                # tail(sg, 8n-1) so histL a=1 sees the final evac of tile
                # n-1 (Tile cannot order against not-yet-issued writes).
                emit_histL(n, sg, 1)
                emit_lhev(sg)

            def queue_boundary_extras(n):
                if n + 1 < nblk:
                    for a in (4, 3, 2):
                        for sg in range(NSTREAM):
                            extras.append(("histL", n + 1, sg, a))
                for sg in range(NSTREAM):
                    extras.append(("out", n - 1, sg))

            # prologue: block 0 prefetch + first front
            for a in (4, 3, 2, 1):
                for sg in range(NSTREAM):
                    emit_histL(0, sg, a)
            for sg in range(NSTREAM):
                emit_lhev(sg)
            for sg in range(NSTREAM):
                nc.tensor.matmul(h_ps[sg][0][:, :], wh1i_t[:],
                                 uh[sg][3][:], start=True, stop=True)
            if nblk > 1:
                for a in (4, 3, 2):
                    for sg in range(NSTREAM):
                        extras.append(("histL", 1, sg, a))
            emit_front(0, 0)

            emit_front(1, 0)
            for sg in range(NSTREAM):
                emit_pcinj(sg)
            for gs in range(steps):
                boundary = gs + 1 < steps and (gs + 1) % T == 0
                emit_tail(0, gs)
                if boundary:
                    boundary_stream(0, (gs + 1) // T)
                emit_front(0, gs + 1)
                if boundary:
                    emit_pcinj(0)
                pop_extras(1)
                emit_tail(1, gs)
                if boundary:
                    boundary_stream(1, (gs + 1) // T)
                    queue_boundary_extras((gs + 1) // T)
                emit_front(1, gs + 1)
                if boundary:
                    emit_pcinj(1)
                pop_extras(1)
            pop_extras(len(extras))

            for sg in range(NSTREAM):
                emit_out(nblk - 1, sg)
    nc.compile()
    return nc


# ----------------------------------------------------------------------------
# entry point
# ----------------------------------------------------------------------------

_NC_CACHE = {}


def kernel(hist_raw, freq_w_real, freq_w_imag, w1, b1, w2, b2):
    hist_raw = np.asarray(hist_raw, np.float32)
    A = compute_lag_matrices(np.asarray(freq_w_real), np.asarray(freq_w_imag))
    wall, b1g, b1g2, dtb2 = build_weights(
        A, np.asarray(w1), np.asarray(b1), np.asarray(w2), np.asarray(b2))
    hist0 = init_history(hist_raw)

    if "nc" not in _NC_CACHE:
        _NC_CACHE["nc"] = build_nc(STEPS)
    nc = _NC_CACHE["nc"]

    in_maps = []
    for core in range(NCORES):
        hc = hist0[core * BPC:(core + 1) * BPC]
        uh0 = pack_uh0(hc)          # (NSTREAM, 4, 128, NCOL)
        uh0w = uh0.transpose(2, 0, 1, 3).reshape(128, -1).astype(np.float32)
        in_maps.append({
            "uh0w": uh0w, "wall": wall, "b1g": b1g, "b1g2": b1g2,
            "dtb2": dtb2,
        })

    trace = os.environ.get("KERNEL_TRACE", "0") == "1"
    try:
        res = run_bass_kernel_spmd(nc, in_maps, core_ids=list(range(NCORES)),
                                   trace=trace)
    except ModuleNotFoundError:
        res = run_bass_kernel_spmd(nc, in_maps, core_ids=list(range(NCORES)))
    _NC_CACHE["last_result"] = res
    if trace and getattr(res, "exec_time_ns", None):
        print(f"HW exec time: {res.exec_time_ns} ns")
    nblk = STEPS // T
    outs = []
    for r in res.results:
        d = np.asarray(r["traj"]).view(np.float32).reshape(
            nblk, NSTREAM, T, 2, C, NCOL)
        # traj[sg*512 + g*256 + col, 8n+s, c] = d[n, sg, s, g, c, col]
        t = d.transpose(1, 3, 5, 0, 2, 4).reshape(BPC, STEPS, C)
        outs.append(t)
    return np.concatenate(outs, axis=0)
